# revision 45
# baseline (speedup 1.0000x reference)
"""Trainium2 Bass kernel for NodeFeatureExtractor.

Key idea: layer 1 of the MLP is linear, so fold W1 into the feature map
before sampling. Per core, precompute on device

    G[p, :] = W1a^T @ map[p, :]   for all 16384 pixels  (fp16, 128-dim)

where map carries the 484 real channels plus synthetic channels encoding
the bias (ones channel), normalized pixel coords (exact under bilinear
interp since the weights sum to 1 and reproduce linear functions) and
dist-to-boundary evaluated on the pixel grid. Per node only a 128-dim
vector is then bilinearly interpolated:

    h1_pre = sum_k w_k * G[p_k] + deg_n * W1[:, 486]

followed by relu, the 128x128 layer-2 matmul (PE, [hid, node] layout so
W2 stays stationary and no output transposes are needed), relu, and a
contiguous store of h^T (host re-transposes). The gather reads a G2
layout (row p = [G[p]; G[p+FW]]) so ONE 1KB SWDGE descriptor per node
covers all 4 bilinear corners; interpolation runs as 9 wide
tensor_tensor ops with stride-0 broadcast weight APs.

Host does data movement + cheap O(N) prep: layout transforms, bilinear
weights/indices (floor/frac), degree bincount (HW dma_scatter_add loses
colliding read-modify-writes, so an exact on-device histogram is not
achievable) and its max-normalization, final transpose + fp32 cast.
"""
import threading
from contextlib import ExitStack

import numpy as np

import bass_rust
import concourse.bass as bass
import concourse.bacc as bacc
import concourse.mybir as mybir
import concourse.tile as tile
from concourse import masks

F32 = mybir.dt.float32
F16 = mybir.dt.float16
I16 = mybir.dt.int16
ALU = mybir.AluOpType
ACTF = mybir.ActivationFunctionType

N_NODES = 200000
N_CORES = 8
HID = 128
FH = FW = 128
NPIX = FH * FW          # 16384
MCH = 512               # padded input channels (484 real + synthetic)
NCH = 1024              # nodes per main-loop chunk


class CFG:
    def __init__(self, n_shard, n_cores, image_size=512.0):
        assert n_shard % NCH == 0
        self.n_shard = n_shard                      # nodes per core (padded)
        self.n_cores = n_cores
        self.pad_n = n_shard * n_cores              # padded total nodes
        self.image_size = float(image_size)


def build_nc(cfg: CFG) -> bass.Bass:
    nc = bacc.Bacc("TRN2", num_devices=cfg.n_cores)
    ns = cfg.n_shard
    npc = ns // 128                                # weight cols (p-major)
    n_chunks = ns // NCH                           # chunks of 1024 nodes

    map_f16 = nc.dram_tensor("map_f16", [MCH, NPIX], F16, kind="ExternalInput")
    w1a = nc.dram_tensor("w1a", [128, 4, HID], F16, kind="ExternalInput")
    w2T = nc.dram_tensor("w2T", [HID, HID], F16, kind="ExternalInput")
    w486r = nc.dram_tensor("w486r", [128, HID], F16, kind="ExternalInput")
    deg_in = nc.dram_tensor("deg_in", [128, npc], F16, kind="ExternalInput")
    b2col = nc.dram_tensor("b2col", [128, 1], F32, kind="ExternalInput")
    idx_in = nc.dram_tensor("idx_in", [128, n_chunks * 64], I16,
                            kind="ExternalInput")
    wts_in = nc.dram_tensor("wts_in", [128, 4, npc], F16, kind="ExternalInput")
    h_outT = nc.dram_tensor("h_outT", [HID, ns], F16, kind="ExternalOutput")

    with tile.TileContext(nc) as tc, ExitStack() as ctx:
        st = ctx.enter_context(tc.tile_pool(name="static", bufs=1))
        dram = ctx.enter_context(tc.tile_pool(name="dram", bufs=1, space="DRAM"))
        mpool = ctx.enter_context(tc.tile_pool(name="mapl", bufs=2))
        gpool = ctx.enter_context(tc.tile_pool(name="gath", bufs=3))
        spool = ctx.enter_context(tc.tile_pool(name="sacc", bufs=2))
        hpool = ctx.enter_context(tc.tile_pool(name="hid1", bufs=2))
        opool = ctx.enter_context(tc.tile_pool(name="outs", bufs=2))
        psA = ctx.enter_context(tc.tile_pool(name="ps_a", bufs=2, space="PSUM"))
        psT = ctx.enter_context(tc.tile_pool(name="ps_t", bufs=2, space="PSUM"))
        psO = ctx.enter_context(tc.tile_pool(name="ps_o", bufs=2, space="PSUM"))

        # ---- static loads
        ident = st.tile([128, 128], F16)
        masks.make_identity(nc, ident[:])
        w1a_sb = st.tile([128, 4, HID], F16)
        nc.sync.dma_start(w1a_sb[:], w1a[:, :, :])
        w2T_sb = st.tile([HID, HID], F16)
        nc.sync.dma_start(w2T_sb[:], w2T[:, :])
        w486_sb = st.tile([128, HID], F16)
        nc.sync.dma_start(w486_sb[:], w486r[:, :])
        deg_sb = st.tile([128, npc], F16)
        nc.sync.dma_start(deg_sb[:], deg_in[:, :])
        b2_sb = st.tile([128, 1], F32)
        nc.sync.dma_start(b2_sb[:], b2col[:, :])
        idx_sb = st.tile([128, n_chunks * 64], I16)
        nc.sync.dma_start(idx_sb[:], idx_in[:, :])
        wts_sb = st.tile([128, 4, npc], F16)
        nc.sync.dma_start(wts_sb[:], wts_in[:, :, :])

        # ---- G precompute: GT[hid, px] = sum_k w1a_k^T @ map_k
        GT = st.tile([128, NPIX], F16)
        TPX = 2048
        for t in range(NPIX // TPX):
            msb = mpool.tile([128, 4, TPX], F16)
            nc.sync.dma_start(
                msb[:], map_f16[:, t * TPX:(t + 1) * TPX]
                .rearrange("(k p) x -> p k x", k=4))
            for pg in range(TPX // 512):
                ps = psA.tile([128, 512], F32, tag="gmm")
                for k in range(4):
                    nc.tensor.matmul(ps[:], w1a_sb[:, k, :],
                                     msb[:, k, pg * 512:(pg + 1) * 512],
                                     start=(k == 0), stop=(k == 3))
                off = t * TPX + pg * 512
                if pg % 2 == 0:
                    nc.scalar.activation(GT[:, off:off + 512], ps[:], ACTF.Copy)
                else:
                    nc.vector.tensor_copy(GT[:, off:off + 512], ps[:])

        # transpose GT -> pixel-major G_pm, then one DMA to HBM
        G_pm = st.tile([128, 128, HID], F16)        # [px%128, px//128, hid]
        for b in range(128):
            pst_ = psA.tile([128, 128], F16, tag="gmm")
            nc.tensor.transpose(pst_[:], GT[:, b * 128:(b + 1) * 128],
                                ident[:])
            if b % 2 == 0:
                nc.scalar.activation(G_pm[:, b, :], pst_[:], ACTF.Copy)
            else:
                nc.vector.tensor_copy(G_pm[:, b, :], pst_[:])

        # G2: row p = [G[p]; G[p+FW]] so one 1KB gather element at row p00
        # covers all 4 bilinear corners (rows p00, p00+1). Written in 16-block
        # groups so the HBM writes overlap the transpose stage above.
        G2_hbm = dram.tile([NPIX, 2 * HID], F16)
        ztail = st.tile([128, HID], F16)
        nc.vector.memset(ztail[:], 0.0)
        for g in range(8):
            r0 = g * 2048
            nc.sync.dma_start(
                G2_hbm[r0:r0 + 2048, 0:HID]
                .rearrange("(b p) h -> p b h", p=128),
                G_pm[:, g * 16:(g + 1) * 16, :])
            nb = 16 if g < 7 else 15
            nc.sync.dma_start(
                G2_hbm[r0:r0 + nb * 128, HID:2 * HID]
                .rearrange("(b p) h -> p b h", p=128),
                G_pm[:, g * 16 + 1:g * 16 + 1 + nb, :])
        nc.sync.dma_start(G2_hbm[NPIX - 128:NPIX, HID:2 * HID], ztail[:])

        gsrc = bass_rust.AP(G2_hbm[:, :].tensor, 0,
                            [[2 * HID, NPIX - 1], [1, 4 * HID]])

        # ---- main loop
        for c in range(n_chunks):
            gall = gpool.tile([128, 8, 4 * HID], F16)
            nc.gpsimd.dma_gather(gall[:], gsrc,
                                 idx_sb[:, c * 64:(c + 1) * 64],
                                 NCH, NCH, 4 * HID, elem_step=2 * HID)

            cs = slice(c * 8, c * 8 + 8)

            def wb(k):
                return (wts_sb[:, k, cs].unsqueeze(2)
                        .to_broadcast([128, 8, HID]))

            g00 = gall[:, :, 0:HID]
            g10 = gall[:, :, HID:2 * HID]
            g01 = gall[:, :, 2 * HID:3 * HID]
            g11 = gall[:, :, 3 * HID:4 * HID]

            s = spool.tile([128, 8, HID], F16)
            t = spool.tile([128, 8, HID], F16, tag="t")
            u = spool.tile([128, 8, HID], F16, tag="u")

            nc.vector.tensor_tensor(
                u[:], deg_sb[:, cs].unsqueeze(2).to_broadcast([128, 8, HID]),
                w486_sb[:].unsqueeze(1).to_broadcast([128, 8, HID]), ALU.mult)
            nc.vector.tensor_tensor(s[:], g00, wb(0), ALU.mult)
            nc.vector.tensor_tensor(t[:], g01, wb(1), ALU.mult)
            nc.vector.tensor_tensor(s[:], s[:], t[:], ALU.add)
            nc.vector.tensor_tensor(t[:], g10, wb(2), ALU.mult)
            nc.vector.tensor_tensor(u[:], u[:], t[:], ALU.add)
            nc.vector.tensor_tensor(t[:], g11, wb(3), ALU.mult)
            nc.vector.tensor_tensor(u[:], u[:], t[:], ALU.add)
            nc.vector.tensor_tensor(s[:], s[:], u[:], ALU.add)

            h1T = hpool.tile([128, 8, 128], F16)
            for half in range(2):
                pt = psT.tile([128, 4, 128], F16, tag="pt")
                for j in range(4):
                    q = half * 4 + j
                    nc.tensor.transpose(pt[:, j, :], s[:, q, :], ident[:])
                nc.scalar.activation(h1T[:, half * 4:half * 4 + 4, :], pt[:],
                                     ACTF.Relu)

            ps2 = psO.tile([128, 2, 512], F32, tag="ps2")
            h1T_flat = h1T[:].rearrange("p q h -> p (q h)")
            for i in range(2):
                nc.tensor.matmul(ps2[:, i, :], w2T_sb[:],
                                 h1T_flat[:, i * 512:(i + 1) * 512],
                                 start=True, stop=True)
            osb = opool.tile([128, NCH], F16)
            nc.scalar.activation(osb[:], ps2[:].rearrange("p i x -> p (i x)"),
                                 ACTF.Relu, bias=b2_sb[:, :])
            nc.sync.dma_start(h_outT[:, NCH * c:NCH * (c + 1)], osb[:])

    nc.compile()
    return nc


# ---------------- host side ----------------

def prep_inputs(cfg: CFG, vertices, backbone_features, seg_probs, edge_index,
                W1, b1, W2, b2):
    """Host prep: layout transforms, bilinear weights/indices, degree."""
    imsz = cfg.image_size
    v = np.asarray(vertices, np.float32)
    n = v.shape[0]
    if n < cfg.pad_n:
        v = np.concatenate([v, np.repeat(v[-1:], cfg.pad_n - n, 0)], 0)

    sx = np.float32((FW - 1) / imsz)
    ix = v[:, 0] * sx
    iy = v[:, 1] * sx
    x0 = np.clip(np.floor(ix), 0, FW - 2).astype(np.float32)
    y0 = np.clip(np.floor(iy), 0, FH - 2).astype(np.float32)
    wx = (ix - x0).astype(np.float32)
    wy = (iy - y0).astype(np.float32)
    w00 = (1 - wx) * (1 - wy)
    w01 = wx * (1 - wy)
    w10 = (1 - wx) * wy
    w11 = wx * wy
    p00 = (y0 * FW + x0).astype(np.int16)

    ep = np.asarray(edge_index).reshape(-1).astype(np.int64)
    degree = np.bincount(ep, minlength=cfg.pad_n).astype(np.float32)
    degn = (degree / (degree.max() + np.float32(1e-6))).astype(np.float32)

    # feature map with synthetic channels, fp16, channel-major
    W1 = np.asarray(W1, np.float32)
    b1 = np.asarray(b1, np.float32)
    m = np.zeros((MCH, NPIX), np.float32)
    m[0:480] = np.asarray(backbone_features, np.float32).reshape(480, NPIX)
    m[480:484] = np.asarray(seg_probs, np.float32).reshape(4, NPIX)
    m[484] = 1.0                                          # -> b1
    xs = (np.arange(FW, dtype=np.float32) / (FW - 1))
    m[485] = np.tile(xs, FH)                              # -> W1[:, 0] (cx)
    m[486] = np.repeat(xs, FW)                            # -> W1[:, 1] (cy)
    gx = xs * np.float32(imsz)                            # vertex-space coord
    dgx = np.minimum(gx, np.float32(imsz) - gx)
    dist = np.minimum(dgx[None, :], dgx[:, None]) / np.float32(imsz / 2)
    m[487] = dist.reshape(-1)                             # -> W1[:, 487]
    map_f16 = m.astype(np.float16)

    w1a = np.zeros((MCH, HID), np.float32)
    w1a[0:480] = W1[:, 2:482].T
    w1a[480:484] = W1[:, 482:486].T
    w1a[484] = b1
    w1a[485] = W1[:, 0]
    w1a[486] = W1[:, 1]
    w1a[487] = W1[:, 487]
    w1a16 = np.ascontiguousarray(
        w1a.reshape(4, 128, HID).transpose(1, 0, 2)).astype(np.float16)
    w2T16 = np.ascontiguousarray(np.asarray(W2, np.float32).T).astype(np.float16)
    w486r = np.ascontiguousarray(
        np.tile(W1[:, 486][None, :], (128, 1))).astype(np.float16)
    b2c = np.asarray(b2, np.float32).reshape(128, 1)

    ns = cfg.n_shard
    npc = ns // 128
    n_chunks = ns // NCH
    in_maps = []
    for c in range(cfg.n_cores):
        sl = slice(c * ns, (c + 1) * ns)

        def pmaj(a):
            # [128, npc] with entry [p, col] = node col*128+p
            return np.ascontiguousarray(a[sl].reshape(npc, 128).T)

        # 16-wrap index layout for dma_gather: [16, ns//16] tiled to 128 rows
        idx16 = np.ascontiguousarray(p00[sl].reshape(-1, 16).T)
        idx_all = np.tile(idx16, (8, 1))

        wts = np.stack([pmaj(w00), pmaj(w01), pmaj(w10), pmaj(w11)], axis=1)
        in_maps.append({
            "map_f16": map_f16, "w1a": w1a16, "w2T": w2T16,
            "w486r": w486r, "b2col": b2c,
            "deg_in": pmaj(degn).astype(np.float16),
            "idx_in": np.ascontiguousarray(idx_all),
            "wts_in": np.ascontiguousarray(wts).astype(np.float16),
        })
    return in_maps


_NC_CACHE: dict = {}
_NC_LOCK = threading.Lock()


def kernel(vertices, backbone_features, seg_probs, edge_index, W1, b1, W2, b2,
           image_size):
    from concourse.bass_utils import run_bass_kernel_spmd

    n = int(np.asarray(vertices).shape[0])
    n_shard = -(-n // (N_CORES * NCH)) * NCH
    cfg = CFG(n_shard, N_CORES, float(np.asarray(image_size)))

    key = (cfg.n_shard, cfg.n_cores, cfg.image_size)
    with _NC_LOCK:
        if key not in _NC_CACHE:
            _NC_CACHE[key] = build_nc(cfg)
        nc = _NC_CACHE[key]

    in_maps = prep_inputs(cfg, vertices, backbone_features, seg_probs,
                          edge_index, W1, b1, W2, b2)

    res = run_bass_kernel_spmd(nc, in_maps, core_ids=list(range(N_CORES)))
    h = np.concatenate(
        [res.results[c]["h_outT"].T for c in range(N_CORES)], 0)
    return np.ascontiguousarray(h[:n]).astype(np.float32)
